# revision 28
# baseline (speedup 1.0000x reference)
"""Trainium2 Bass kernel: GNN attention message-passing layer.

Computes, for full inputs:
    y   = x @ weight                      # [N, OD]
    ev  = values * attn[row, col]         # [E]
    out = segment_sum(ev[:,None] * y[col], row, N)   # [N, OD]
    nrm = ||attn||_F
Returns (out, nrm) matching the reference.

Sharding: rows of x / attn / output are owner-partitioned across 8 cores
(1024 rows each); the edge list is partitioned by destination row; weight
and y (2 MB) are replicated for phase 2. Two SPMD launches:
  phase 1: y_d = x_d @ W        (x pre-transposed on host so the
           contraction dim lands on SBUF partitions)
  phase 2: local attn gather + y-row gather + segment-sum + attn sumsq
"""

import os
import numpy as np

from concourse import bass, bacc, mybir
import concourse.tile as tile
from concourse.bass_utils import run_bass_kernel_spmd
from concourse.masks import make_identity
from concourse.tile_rust import add_dep_helper

N = 8192
E = 262144
OD = 64
NCORES = 8
P = 128
RPD = N // NCORES          # rows per device (1024)
NBLK = RPD // P            # 128-row blocks per device (8)

F32 = mybir.dt.float32
BF16 = mybir.dt.bfloat16
F32R = mybir.dt.float32r
I32 = mybir.dt.int32
I16 = mybir.dt.int16

LAST_PROFILE = {}


# --------------------------------------------------------------------------
# phase 1: y = x @ W   (per core: xT [n, rpd] , w [n, od] -> y [rpd, od])
# --------------------------------------------------------------------------
def build_phase1(n=N, rpd=RPD, od=OD):
    nk = n // P
    nm = rpd // 512
    nc = bacc.Bacc("TRN2")
    xT = nc.declare_dram_parameter("xT", [n, rpd], F32, isOutput=False)
    w = nc.declare_dram_parameter("w", [n, od], F32, isOutput=False)
    y = nc.declare_dram_parameter("y", [rpd, od], F32, isOutput=True)
    with tile.TileContext(nc) as tc:
        with tc.tile_pool(name="const", bufs=1) as cpool, \
             tc.tile_pool(name="xin", bufs=4) as xpool, \
             tc.tile_pool(name="acc", bufs=1, space="PSUM") as pspool, \
             tc.tile_pool(name="tp", bufs=2, space="PSUM") as pstp, \
             tc.tile_pool(name="yout", bufs=2) as ypool:
            w_all = cpool.tile([P, nk, od], F32R)
            nc.sync.dma_start(
                out=w_all[:],
                in_=w[:, :].rearrange("(k p) n -> p k n", p=P).bitcast(F32R))
            ident = cpool.tile([P, P], F32)
            make_identity(nc, ident[:])

            # PE warmup consuming w_all: absorbs the w-load DMA wait on the PE
            # clock so the first accumulation matmul needs only one sync wait
            # (hardware S3_LW decode allows a single wait command).
            warm = pstp.tile([od, od], F32, tag="warm")
            warm_mm = nc.tensor.matmul(out=warm[:], lhsT=w_all[:, 0, :],
                                       rhs=w_all[:, 0, :], start=True, stop=True)

            psums = [pspool.tile([od, 512], F32, name=f"acc{h}", tag=f"acc{h}")
                     for h in range(nm)]
            first_mm = None
            for k in range(nk):
                xt = xpool.tile([P, rpd], F32R, tag="xt")
                nc.sync.dma_start(out=xt[:],
                                  in_=xT[k * P:(k + 1) * P, :].bitcast(F32R))
                for h in range(nm):
                    mm = nc.tensor.matmul(
                        out=psums[h][:],
                        lhsT=w_all[:, k, :],
                        rhs=xt[:, h * 512:(h + 1) * 512],
                        start=(k == 0), stop=(k == nk - 1))
                    if first_mm is None:
                        first_mm = mm
                        add_dep_helper(first_mm.ins, warm_mm.ins,
                                       reason="warmup before first matmul")

            yT_sb = ypool.tile([od, rpd], F32, tag="yT")
            for h in range(nm):
                nc.vector.tensor_copy(out=yT_sb[:, h * 512:(h + 1) * 512],
                                      in_=psums[h][:])
            for q in range(rpd // P):
                pt = pstp.tile([P, od], F32, tag="pt")
                nc.tensor.transpose(out=pt[:], in_=yT_sb[:, q * P:(q + 1) * P],
                                    identity=ident[:od, :od])
                ysb = ypool.tile([P, od], F32, tag="yo")
                nc.vector.tensor_copy(out=ysb[:], in_=pt[:])
                nc.sync.dma_start(out=y[q * P:(q + 1) * P, :], in_=ysb[:])
    nc.finalize()
    return nc


# --------------------------------------------------------------------------
# phase 2: gather + segment-sum + attn norm
#   attn [rpd, n], y [n, od] (full), per-block packed edge tensors,
#   -> out [rpd, od], ssq [128, 1] (per-partition attn sum-of-squares)
# --------------------------------------------------------------------------
def build_phase2(eb, rpd=RPD, n=N, od=OD, nblk=NBLK,
                 use_indirect=True, use_gather=True, use_norm=True):
    nj = eb // P
    s16 = eb // 16
    nc = bacc.Bacc("TRN2")
    attn = nc.declare_dram_parameter("attn", [rpd, n], F32, isOutput=False)
    y = nc.declare_dram_parameter("y", [n, od], F32, isOutput=False)
    colw = nc.declare_dram_parameter("colw", [nblk, P, s16], I16, isOutput=False)
    acix = nc.declare_dram_parameter("acix", [nblk, P, s16], I16, isOutput=False)
    aoff = nc.declare_dram_parameter("aoff", [nblk, P, nj], F32, isOutput=False)
    vals = nc.declare_dram_parameter("vals", [nblk, P, nj], F32, isOutput=False)
    rowl = nc.declare_dram_parameter("rowl", [nblk, P, nj], F32, isOutput=False)
    iot = nc.declare_dram_parameter("iot", [P, P], F32, isOutput=False)
    out = nc.declare_dram_parameter("out", [rpd, od], F32, isOutput=True)
    ssq = nc.declare_dram_parameter("ssq", [P, 1], F32, isOutput=True)
    with tile.TileContext(nc) as tc:
        with tc.tile_pool(name="const", bufs=1) as cpool, \
             tc.tile_pool(name="idx", bufs=2) as ipool, \
             tc.tile_pool(name="gath", bufs=2) as gpool, \
             tc.tile_pool(name="ev", bufs=2) as epool, \
             tc.tile_pool(name="mask", bufs=3) as mpool, \
             tc.tile_pool(name="anorm", bufs=3) as apool, \
             tc.tile_pool(name="pacc", bufs=4) as accpool, \
             tc.tile_pool(name="po", bufs=2, space="PSUM") as pspool, \
             tc.tile_pool(name="outp", bufs=2) as opool:
            iot_f = cpool.tile([P, P], F32)
            nc.sync.dma_start(out=iot_f[:], in_=iot[:, :])
            norm_acc = cpool.tile([P, 1], F32)
            nc.vector.memset(norm_acc[:], 0.0)

            # Edge tensors are small — load them entirely up front so no
            # SBUF slot is ever reloaded by a HWDGE DMA (keeps every HWDGE
            # DMA at a single sync wait).
            # dma_gather ucode requires a compact [128, num_idxs/16] index
            # tensor (a strided slice of a larger tile crashes on HW), so
            # load one dedicated tile per block.
            colts = []
            acixs = []
            for b in range(nblk):
                ct = cpool.tile([P, s16], I16, name=f"colt{b}", tag=f"colt{b}")
                nc.sync.dma_start(out=ct[:], in_=colw[b])
                colts.append(ct)
                at2 = cpool.tile([P, s16], I16, name=f"acix{b}", tag=f"acix{b}")
                nc.sync.dma_start(out=at2[:], in_=acix[b])
                acixs.append(at2)
            aoff_all = cpool.tile([P, nblk, nj], F32)
            nc.sync.dma_start(out=aoff_all[:],
                              in_=aoff[:, :, :].rearrange("b p s -> p b s"))
            vt_all = cpool.tile([P, nblk, nj], F32)
            nc.sync.dma_start(out=vt_all[:],
                              in_=vals[:, :, :].rearrange("b p s -> p b s"))
            rt_all = cpool.tile([P, nblk, nj], F32)
            nc.sync.dma_start(out=rt_all[:],
                              in_=rowl[:, :, :].rearrange("b p s -> p b s"))

            for b in range(nblk):
                yg = gpool.tile([P, nj, od], F32, tag="yg")
                if use_gather:
                    # dma_gather ucode caps at 1024 indices per instruction
                    for o in range(0, eb, 1024):
                        sz = min(1024, eb - o)
                        nc.gpsimd.dma_gather(
                            out_ap=yg[:, o // P:(o + sz) // P, :],
                            in_ap=y[:, :],
                            idxs_ap=colts[b][:, o // 16:(o + sz) // 16],
                            num_idxs=sz, num_idxs_reg=sz, elem_size=od)
                else:
                    nc.vector.memset(yg[:], 1.0)

                ev = epool.tile([P, nj], F32, tag="ev")
                if use_indirect:
                    # gather the 64-float chunk holding each attn[row, col]
                    q = b // 2
                    attn_q = attn[q * 2 * P:(q + 1) * 2 * P, :].rearrange(
                        "r (c d) -> (r c) d", d=64)
                    ac = gpool.tile([P, nj, od], F32, tag="ac")
                    for o in range(0, eb, 1024):
                        sz = min(1024, eb - o)
                        nc.gpsimd.dma_gather(
                            out_ap=ac[:, o // P:(o + sz) // P, :],
                            in_ap=attn_q,
                            idxs_ap=acixs[b][:, o // 16:(o + sz) // 16],
                            num_idxs=sz, num_idxs_reg=sz, elem_size=64)
                    # extract element (col % 64) via iota-compare + reduce
                    msk = gpool.tile([P, nj, od], F32, tag="msk")
                    nc.vector.tensor_tensor(
                        out=msk[:],
                        in0=aoff_all[:, b, :, None].to_broadcast([P, nj, od]),
                        in1=iot_f[:, None, 0:od].to_broadcast([P, nj, od]),
                        op=mybir.AluOpType.is_equal)
                    nc.vector.tensor_mul(out=msk[:], in0=msk[:], in1=ac[:])
                    av = epool.tile([P, nj], F32, tag="av")
                    nc.vector.tensor_reduce(
                        out=av[:], in_=msk[:], axis=mybir.AxisListType.X,
                        op=mybir.AluOpType.add)
                    nc.vector.tensor_mul(out=ev[:], in0=av[:],
                                         in1=vt_all[:, b, :])
                else:
                    nc.vector.tensor_copy(out=ev[:], in_=vt_all[:, b, :])
                ys = gpool.tile([P, nj, od], BF16, tag="ys")
                nc.vector.tensor_mul(out=ys[:], in0=yg[:],
                                     in1=ev[:, :, None].to_broadcast([P, nj, od]))

                rt = rt_all[:, b, :]
                mk = mpool.tile([P, nj, P], BF16, tag="mk")
                nc.vector.tensor_tensor(
                    out=mk[:], in0=rt[:, :, None].to_broadcast([P, nj, P]),
                    in1=iot_f[:, None, :].to_broadcast([P, nj, P]),
                    op=mybir.AluOpType.is_equal)
                po = pspool.tile([P, od], F32, tag="po")
                for c in range(nj):
                    nc.tensor.matmul(out=po[:], lhsT=mk[:, c, :],
                                     rhs=ys[:, c, :],
                                     start=(c == 0), stop=(c == nj - 1))
                ot = opool.tile([P, od], F32, tag="ot")
                nc.vector.tensor_copy(out=ot[:], in_=po[:])
                nc.sync.dma_start(out=out[b * P:(b + 1) * P, :], in_=ot[:])

            # attn Frobenius-norm partials: square+row-accumulate on ACT
            ftile = min(4096, n)
            for g in range(rpd // P if use_norm else 0):
                for f in range(n // ftile):
                    at = apool.tile([P, ftile], F32, tag="at")
                    nc.sync.dma_start(
                        out=at[:], in_=attn[g * P:(g + 1) * P,
                                            f * ftile:(f + 1) * ftile])
                    sq = apool.tile([P, ftile], F32, tag="sq")
                    pacc = accpool.tile([P, 1], F32, tag="pc")
                    nc.scalar.activation(
                        out=sq[:], in_=at[:],
                        func=mybir.ActivationFunctionType.Square,
                        accum_out=pacc[:])
                    nc.vector.tensor_add(out=norm_acc[:], in0=norm_acc[:],
                                         in1=pacc[:])
            nc.sync.dma_start(out=ssq[:], in_=norm_acc[:])
    nc.finalize()
    return nc


# --------------------------------------------------------------------------
# host-side edge packing
# --------------------------------------------------------------------------
def prep_edges(row, col, values, n=N, rpd=RPD, ncores=NCORES, nblk=NBLK):
    order = np.argsort(row, kind="stable")
    rs = row[order]
    cs = col[order]
    vs = values[order]
    nb_total = n // P
    blk = rs // P
    counts = np.bincount(blk, minlength=nb_total)
    eb = max(int(counts.max()), P)
    eb = ((eb + P - 1) // P) * P
    nj = eb // P
    starts = np.zeros(nb_total + 1, np.int64)
    starts[1:] = np.cumsum(counts)

    colw = np.zeros((ncores, nblk, P, eb // 16), np.int16)
    acix = np.zeros((ncores, nblk, P, eb // 16), np.int16)
    aoff = np.zeros((ncores, nblk, P, nj), np.float32)
    vals = np.zeros((ncores, nblk, P, nj), np.float32)
    rowl = np.full((ncores, nblk, P, nj), -1.0, np.float32)
    qrows = 2 * P  # attn quarter height: chunk ids stay within int16
    for d in range(ncores):
        for b in range(nblk):
            g = d * nblk + b
            s, e = int(starts[g]), int(starts[g + 1])
            cnt = e - s
            i = np.arange(cnt)
            cw = np.zeros((16, eb // 16), np.int16)
            cw[i % 16, i // 16] = cs[s:e].astype(np.int16)
            colw[d, b] = np.tile(cw, (8, 1))
            # chunk id of attn[row, col] within the 256-row quarter that
            # contains block b: (row - quarter_base)*(n/64) + col//64
            qbase = d * rpd + (b // 2) * qrows
            cid = ((rs[s:e] - qbase) * (n // 64) + cs[s:e] // 64)
            aw = np.zeros((16, eb // 16), np.int16)
            aw[i % 16, i // 16] = cid.astype(np.int16)
            acix[d, b] = np.tile(aw, (8, 1))
            aoff[d, b, i % P, i // P] = (cs[s:e] % 64).astype(np.float32)
            vals[d, b, i % P, i // P] = vs[s:e]
            rowl[d, b, i % P, i // P] = (rs[s:e] - (d * rpd + b * P)
                                         ).astype(np.float32)
    return eb, colw, acix, aoff, vals, rowl


# --------------------------------------------------------------------------
# execution helpers
# --------------------------------------------------------------------------
def _run(nc, in_maps, trace=False):
    if os.environ.get("BASS_KERNEL_BACKEND") == "sim":
        from concourse import bass_interp
        results = []
        for m in in_maps:
            sim = bass_interp.CoreSim(nc)
            for k, v in m.items():
                sim.tensor(k)[:] = v
            sim.simulate()
            out = {}
            for alloc in nc.m.functions[0].allocations:
                if not isinstance(alloc, mybir.MemoryLocationSet):
                    continue
                if alloc.kind == "ExternalOutput":
                    name = alloc.memorylocations[0].name
                    out[name] = np.array(sim.tensor(name))
            results.append(out)
        return results, None
    res = run_bass_kernel_spmd(nc, in_maps, list(range(len(in_maps))),
                               trace=trace)
    return res.results, res


def kernel(**inputs):
    x = np.ascontiguousarray(np.asarray(inputs["x"], dtype=np.float32))
    attn = np.ascontiguousarray(np.asarray(inputs["attn"], dtype=np.float32))
    weight = np.ascontiguousarray(np.asarray(inputs["weight"], dtype=np.float32))
    values = np.ascontiguousarray(np.asarray(inputs["values"], dtype=np.float32))
    row = np.asarray(inputs["row"]).astype(np.int64)
    col = np.asarray(inputs["col"]).astype(np.int64)
    trace = os.environ.get("BASS_KERNEL_TRACE") == "1"

    # phase 1
    nc1 = build_phase1()
    in_maps1 = [{"xT": np.ascontiguousarray(x[d * RPD:(d + 1) * RPD, :].T),
                 "w": weight} for d in range(NCORES)]
    res1, raw1 = _run(nc1, in_maps1, trace=trace)
    y_full = np.ascontiguousarray(
        np.concatenate([res1[d]["y"] for d in range(NCORES)], axis=0))

    # phase 2
    eb, colw, acix, aoff, vals, rowl = prep_edges(row, col, values)
    nc2 = build_phase2(eb)
    iot = np.tile(np.arange(P, dtype=np.float32), (P, 1))
    in_maps2 = [{"attn": attn[d * RPD:(d + 1) * RPD, :],
                 "y": y_full,
                 "colw": colw[d], "acix": acix[d], "aoff": aoff[d],
                 "vals": vals[d], "rowl": rowl[d], "iot": iot}
                for d in range(NCORES)]
    res2, raw2 = _run(nc2, in_maps2, trace=trace)

    out_full = np.concatenate([res2[d]["out"] for d in range(NCORES)], axis=0)
    sumsq = np.float64(0.0)
    for d in range(NCORES):
        sumsq += res2[d]["ssq"].astype(np.float64).sum()
    nrm = np.float32(np.sqrt(sumsq))

    LAST_PROFILE.clear()
    for name, raw in (("phase1", raw1), ("phase2", raw2)):
        if raw is not None:
            LAST_PROFILE[name] = {
                "exec_time_ns": raw.exec_time_ns,
                "profile_json": raw.profile_json,
            }
    return out_full, nrm
